# revision 19
# baseline (speedup 1.0000x reference)
"""Trainium2 Bass kernel for a dense pre-LN transformer block.

Sharding: 8 cores = 4 batches x 2 sequence-halves (zigzag query blocks).
Each core handles one batch element; K/V are computed redundantly for the
full sequence on both cores of a batch, and each core computes attention +
proj + FFN for 1024 of the 2048 query tokens.

Precision scheme (validated offline vs the fp32 reference, rel err 9e-3):
- All 128-contraction matmuls (QKV, proj, FFN1, FFN2) run as fp8e4
  DoubleRow with a hi+lo split of both operands, keeping 3 of the 4
  cross terms: x@W ~= xh@Wh + xl@Wh + xh@Wl.  DoubleRow processes two
  128-row contraction slices per instruction at ~3.5x the bf16 rate, so
  the 3-term split is ~2.3x faster than bf16 at bf16-level accuracy.
- Scores: q stored as plain fp8 (duplicated into both DoubleRow slices),
  k stored as hi+lo -> one DoubleRow matmul per (head, chunk).
- ctx (attn @ V), softmax, LN: bf16/f32.
Weights are pre-scaled by 16 host-side (fp8 subnormal avoidance); the
descale folds into psum-evacuation scales; V's 16x folds away entirely
via the softmax normalizer (Z scales too).
"""

import contextlib

import numpy as np
import ml_dtypes

from concourse import bass, bacc, tile, mybir
from concourse.bass_utils import run_bass_kernel_spmd

F32 = mybir.dt.float32
BF16 = mybir.dt.bfloat16
F8 = mybir.dt.float8e4
E4 = ml_dtypes.float8_e4m3
BF = ml_dtypes.bfloat16
DR = mybir.MatmulPerfMode.DoubleRow
AF = mybir.ActivationFunctionType

B, T, D = 4, 2048, 1024
H, HD = 16, 64
DFF = 4 * D
EPS = 1e-5
N_CORES = 8
WS = 16.0          # host-side weight prescale
FFS = 4.0          # ffT activation prescale
CXS = 4.0          # ctxT activation prescale

FULL_CFG = dict(D=1024, H=16, T=2048, QB=512, DFF=4096, NG=4)
ABLATE = set()  # sim-diagnostic cost-shrink switches


def _ab(tag, sl):
    """Shrink a column slice to 8 wide when `tag` is ablated (sim only)."""
    if tag in ABLATE:
        return slice(sl.start or 0, (sl.start or 0) + 8)
    return sl


def derive(cfg):
    c = dict(cfg)
    c["DC"] = cfg["D"] // 128            # d-chunks
    c["DP"] = c["DC"] // 2               # d-chunk pairs
    c["FC"] = cfg["H"] * HD // 128       # feature chunks (head pairs)
    c["FCP"] = 2                         # f-chunks per pass
    c["NPASS"] = c["FC"] // c["FCP"]
    c["S"] = cfg["T"] // 128             # key chunks
    c["QBC"] = cfg["QB"] // 128          # chunks per query block
    c["NT"] = cfg["QB"]                  # moving-dim tile (== query block)
    c["TOWN"] = 2 * cfg["QB"]            # tokens owned per core
    c["TOC"] = c["TOWN"] // 128
    c["NO"] = min(512, cfg["D"])
    c["OC"] = cfg["D"] // c["NO"]        # dout chunks of <=512
    c["GFC"] = (cfg["DFF"] // cfg["NG"]) // 128  # f-chunks per FFN group
    c["GFP"] = c["GFC"] // 2             # f-chunk pairs per group
    c["KTB"] = cfg["T"] // c["NT"]       # t-blocks for k over full T
    return c


def build(cfg):
    """Emit the bass program for one core. Returns nc."""
    c = derive(cfg)
    Dm, Tf, DFFm, NG = cfg["D"], cfg["T"], cfg["DFF"], cfg["NG"]
    DC, DP, FC, FCP, NPASS = c["DC"], c["DP"], c["FC"], c["FCP"], c["NPASS"]
    S, QBC, NT, TOWN, TOC = c["S"], c["QBC"], c["NT"], c["TOWN"], c["TOC"]
    OC, NO, GFC, GFP, KTB = c["OC"], c["NO"], c["GFC"], c["GFP"], c["KTB"]
    HDf = HD  # 64
    VW = FCP * 130  # v columns per pass (2 heads x (64+1), x2)

    nc = bacc.Bacc("TRN2", target_bir_lowering=False, debug=False)

    # ---- DRAM I/O ----
    x_d = nc.dram_tensor("x", [Tf, Dm], F32, kind="ExternalInput")
    wqh_d = nc.dram_tensor("wqh", [NPASS, DP, 128, 2, FCP * 128], F8,
                           kind="ExternalInput")
    wql_d = nc.dram_tensor("wql", [NPASS, DP, 128, 2, FCP * 128], F8,
                           kind="ExternalInput")
    wkh_d = nc.dram_tensor("wkh", [NPASS, DP, 128, 2, FCP * 128], F8,
                           kind="ExternalInput")
    wkl_d = nc.dram_tensor("wkl", [NPASS, DP, 128, 2, FCP * 128], F8,
                           kind="ExternalInput")
    wvh_d = nc.dram_tensor("wvh", [NPASS, DP, 128, 2, VW], F8,
                           kind="ExternalInput")
    wvl_d = nc.dram_tensor("wvl", [NPASS, DP, 128, 2, VW], F8,
                           kind="ExternalInput")
    bq_d = nc.dram_tensor("bq", [128, FC], F32, kind="ExternalInput")
    bk_d = nc.dram_tensor("bk", [128, FC], F32, kind="ExternalInput")
    bv_d = nc.dram_tensor("bv", [NPASS, 1, VW], F32, kind="ExternalInput")
    woh_d = nc.dram_tensor("woh", [FC // 2, 128, 2, Dm], F8,
                           kind="ExternalInput")
    wol_d = nc.dram_tensor("wol", [FC // 2, 128, 2, Dm], F8,
                           kind="ExternalInput")
    bo_d = nc.dram_tensor("bo", [1, Dm], F32, kind="ExternalInput")
    w1h_d = nc.dram_tensor("w1h", [NG, DP, 128, 2, DFFm // NG], F8,
                           kind="ExternalInput")
    w1l_d = nc.dram_tensor("w1l", [NG, DP, 128, 2, DFFm // NG], F8,
                           kind="ExternalInput")
    b1_d = nc.dram_tensor("b1", [128, DFFm // 128], F32,
                          kind="ExternalInput")
    w2h_d = nc.dram_tensor("w2h", [NG, GFP, 128, 2, Dm], F8,
                           kind="ExternalInput")
    w2l_d = nc.dram_tensor("w2l", [NG, GFP, 128, 2, Dm], F8,
                           kind="ExternalInput")
    b2_d = nc.dram_tensor("b2", [128, Dm], F32, kind="ExternalInput")
    tri_d = nc.dram_tensor("tri", [QBC, 128, NT], BF16, kind="ExternalInput")
    cm_d = nc.dram_tensor("cm", [2 * QBC, 128, 1], F32, kind="ExternalInput")
    idn_d = nc.dram_tensor("ident", [128, 128], BF16, kind="ExternalInput")
    zro_d = nc.dram_tensor("zeros", [128, 1], F32, kind="ExternalInput")
    out_d = nc.dram_tensor("out", [TOWN, Dm], F32, kind="ExternalOutput")

    xr = x_d.ap().rearrange("(n p) d -> n p d", p=128)
    outr = out_d.ap().rearrange("(n p) d -> n p d", p=128)

    with tile.TileContext(nc) as tc, contextlib.ExitStack() as top:
        cpool = top.enter_context(tc.tile_pool(name="const", bufs=1))
        ident = cpool.tile([128, 128], BF16, name="ident", tag="ident")
        nc.sync.dma_start(ident[:], idn_d.ap())
        cms = cpool.tile([128, 2 * QBC], F32, name="cms", tag="cms")
        for i in range(2 * QBC):
            nc.gpsimd.dma_start(cms[:, i:i + 1], cm_d.ap()[i])
        zbias = cpool.tile([128, 1], F32, name="zbias", tag="zbias")
        nc.gpsimd.dma_start(zbias[:], zro_d.ap())
        bqall = cpool.tile([128, FC], F32, name="bqall", tag="bqall")
        nc.gpsimd.dma_start(bqall[:], bq_d.ap())
        bkall = cpool.tile([128, FC], F32, name="bkall", tag="bkall")
        nc.gpsimd.dma_start(bkall[:], bk_d.ap())

        # x1 residual (+b2 prefolded) stays SBUF-resident for the FFN tail
        x1pool = top.enter_context(tc.tile_pool(name="x1p", bufs=1))
        x1b = [x1pool.tile([128, Dm], BF16, name=f"x1b{t}", tag=f"x1b{t}")
               for t in range(TOC)]
        b2t = cpool.tile([128, Dm], F32, name="b2t", tag="b2t")
        nc.sync.dma_start(b2t[:], b2_d.ap())

        h2_stack = contextlib.ExitStack()
        h2p = h2_stack.enter_context(tc.tile_pool(name="h2Tp", bufs=1))
        h2ha = h2p.tile([128, DP, 2, TOWN], F8, name="h2ha", tag="h2ha")
        h2la = h2p.tile([128, DP, 2, TOWN], F8, name="h2la", tag="h2la")
        h2h = [h2ha[:, j] for j in range(DP)]
        h2l = [h2la[:, j] for j in range(DP)]

        ctx_stack = contextlib.ExitStack()
        ctxp = ctx_stack.enter_context(tc.tile_pool(name="ctxTp", bufs=1))
        cth = [ctxp.tile([128, 2, TOWN], F8, name=f"cth{j}", tag=f"cth{j}")
               for j in range(FC // 2)]
        ctl = [ctxp.tile([128, 2, TOWN], F8, name=f"ctl{j}", tag=f"ctl{j}")
               for j in range(FC // 2)]

        hT_stack = contextlib.ExitStack()
        hp = hT_stack.enter_context(tc.tile_pool(name="hTp", bufs=1))
        hTha = hp.tile([128, DP, 2, Tf], F8, name="hTha", tag="hTha")
        hTla = hp.tile([128, DP, 2, Tf], F8, name="hTla", tag="hTla")
        hTh = [hTha[:, j] for j in range(DP)]
        hTl = [hTla[:, j] for j in range(DP)]
        trip = hT_stack.enter_context(tc.tile_pool(name="trip", bufs=1))
        tri = []
        for i in range(QBC):
            m = trip.tile([128, NT], BF16, name=f"tri{i}", tag=f"tri{i}")
            nc.gpsimd.dma_start(m[:], tri_d.ap()[i])
            tri.append(m)

        # ---------------- Phase 1: LN1 + transpose -> hT hi/lo ----------
        with tc.tile_pool(name="ln1", bufs=4) as lp, \
             tc.tile_pool(name="ln1s", bufs=8) as lsp, \
             tc.tile_pool(name="ln1p", bufs=6, space=bass.MemorySpace.PSUM) as lpp:
            for ti in range(S):
                xt = lp.tile([128, Dm], F32, name="xt", tag="xt")
                nsub = max(1, Dm // 512)
                st6 = lsp.tile([128, nsub, 6], F32, name="st6", tag="st6")
                for sb_i in range(nsub):
                    cs = slice(sb_i * (Dm // nsub), (sb_i + 1) * (Dm // nsub))
                    nc.sync.dma_start(xt[:, cs], xr[ti][:, cs])
                    nc.vector.bn_stats(st6[:, sb_i, :], xt[:, cs])
                agg = lsp.tile([128, 2], F32, name="agg", tag="agg")
                nc.vector.bn_aggr(agg[:], st6[:])
                veps = lsp.tile([128, 1], F32, name="veps", tag="veps")
                nc.vector.tensor_scalar_add(veps[:], agg[:, 1:2], EPS)
                std = lsp.tile([128, 1], F32, name="std", tag="std")
                nc.scalar.sqrt(std[:], veps[:])
                rstd = lsp.tile([128, 1], F32, name="rstd", tag="rstd")
                nc.vector.reciprocal(rstd[:], std[:])
                nmr = lsp.tile([128, 1], F32, name="nmr", tag="nmr")
                nc.vector.tensor_scalar(nmr[:], agg[:, 0:1], rstd[:], -1.0,
                                        op0=mybir.AluOpType.mult,
                                        op1=mybir.AluOpType.mult)
                ht = lp.tile([128, Dm], BF16, name="ht", tag="ht")
                nc.scalar.activation(ht[:], xt[:], AF.Identity,
                                     bias=nmr[:], scale=rstd[:])
                ps = lpp.tile([128, DP, 2, 128], BF16, name="tps", tag="tps")
                for dc in range(DC):
                    nc.tensor.transpose(
                        ps[:, dc // 2, dc % 2, :],
                        ht[:, dc * 128:(dc + 1) * 128], ident[:])
                tc_sl = slice(ti * 128, (ti + 1) * 128)
                dst_h = hTha[:, :, :, tc_sl]
                dst_l = hTla[:, :, :, tc_sl]
                nc.scalar.copy(dst_h, ps[:])
                nc.vector.tensor_tensor(dst_l, ps[:], dst_h,
                                        op=mybir.AluOpType.subtract)

        # ---------------- Phase 2: per-pass QKV + attention -------------
        with tc.tile_pool(name="pass_sb", bufs=1) as pp, \
             tc.tile_pool(name="vtp", bufs=1) as vp, \
             tc.tile_pool(name="wvres", bufs=1) as wvp, \
             tc.tile_pool(name="expp", bufs=4) as ep, \
             tc.tile_pool(name="zrowp", bufs=2) as zp, \
             tc.tile_pool(name="zbp", bufs=2) as zbp, \
             tc.tile_pool(name="kevp", bufs=4) as kev, \
             tc.tile_pool(name="qkvps", bufs=2, space=bass.MemorySpace.PSUM) as qps, \
             tc.tile_pool(name="scps", bufs=2, space=bass.MemorySpace.PSUM) as sps, \
             tc.tile_pool(name="ctxps", bufs=1, space=bass.MemorySpace.PSUM) as cps:
            for p in range(NPASS):
                fcs = [p * FCP + i for i in range(FCP)]
                # kT: [2 heads x 64 feats, hi/lo, T]; qT: same rows, q8 dup'd
                kT = [pp.tile([128, 2, Tf], F8, name=f"kT{i}", tag=f"kT{i}")
                      for i in range(FCP)]
                qT = [pp.tile([128, 2, 2 * NT], F8, name=f"qT{i}",
                              tag=f"qT{i}") for i in range(FCP)]
                bks = [bkall[:, fc:fc + 1] for fc in fcs]
                bqs = [bqall[:, fc:fc + 1] for fc in fcs]
                wk_h, wk_l, wq_h, wq_l = [], [], [], []
                for dp_i in range(DP):
                    for lst, dram in ((wk_h, wkh_d), (wk_l, wkl_d),
                                      (wq_h, wqh_d), (wq_l, wql_d)):
                        t = pp.tile([128, 2, FCP * 128], F8,
                                    name=f"w{len(lst)}_{id(dram) % 97}",
                                    tag=f"w{dram.name}{dp_i}")
                        nc.gpsimd.dma_start(t[:], dram.ap()[p, dp_i])
                        lst.append(t)

                def qk_psum(ps_t, wh, wl, ws, tb0, tbn):
                    if 'qkvmm' in ABLATE:
                        tbn = tb0 + 8
                        ps_t = ps_t[:, 0:8]
                    mm = []
                    for dp_i in range(DP):
                        mm.append((wh[dp_i], hTh[dp_i]))
                    for dp_i in range(DP):
                        mm.append((wl[dp_i], hTh[dp_i]))
                    for dp_i in range(DP):
                        mm.append((wh[dp_i], hTl[dp_i]))
                    for mi, (wt, ht_t) in enumerate(mm):
                        nc.tensor.matmul(
                            ps_t[:], wt[:, :, ws],
                            ht_t[:, :, tb0:tbn],
                            start=(mi == 0), stop=(mi == len(mm) - 1),
                            perf_mode=DR)

                for i, fc in enumerate(fcs):
                    ws = slice(i * 128, (i + 1) * 128)
                    for tb in range(KTB):
                        pk = qps.tile([128, NT], F32, name="pk", tag="qkv")
                        qk_psum(pk, wk_h, wk_l, ws, tb * NT, (tb + 1) * NT)
                        # k_hi (Act), k_lo = (psum/16+bk) - k_hi (DVE x2)
                        kf = kev.tile([128, NT], BF16, name="kf", tag="kf")
                        nc.vector.tensor_scalar(
                            kf[:], pk[:], 1.0 / WS, bks[i][:],
                            op0=mybir.AluOpType.mult,
                            op1=mybir.AluOpType.add)
                        dst_h = kT[i][:, 0, tb * NT:(tb + 1) * NT]
                        dst_l = kT[i][:, 1, tb * NT:(tb + 1) * NT]
                        nc.gpsimd.tensor_copy(dst_h, kf[:])
                        if 'losub' not in ABLATE:
                            nc.vector.tensor_tensor(dst_l, kf[:], dst_h,
                                                    op=mybir.AluOpType.subtract)
                    for tb in range(2):
                        pq = qps.tile([128, NT], F32, name="pq", tag="qkv")
                        qk_psum(pq, wq_h, wq_l, ws, tb * NT, (tb + 1) * NT)
                        for s in range(2):
                            nc.scalar.activation(
                                qT[i][:, s, tb * NT:(tb + 1) * NT], pq[:],
                                AF.Identity, bias=bqs[i][:], scale=1.0 / WS)
                # --- V (token-major, scaled by 16; Z-normalizer absorbs) ---
                bvr = pp.tile([1, VW], F32, name="bvr", tag="bvr")
                nc.sync.dma_start(bvr[:], bv_d.ap()[p])
                bvb = pp.tile([128, VW], F32, name="bvb", tag="bvb")
                nc.gpsimd.partition_broadcast(bvb[:], bvr[:])
                wv_h, wv_l = [], []
                for dp_i in range(DP):
                    for lst, dram in ((wv_h, wvh_d), (wv_l, wvl_d)):
                        t = wvp.tile([128, 2, VW], F8, name=f"wv{dp_i}",
                                     tag=f"wv{dram.name}{dp_i}")
                        nc.gpsimd.dma_start(t[:], dram.ap()[p, dp_i])
                        lst.append(t)
                vt = [vp.tile([128, VW], BF16, name=f"v{ti}",
                              tag=f"v{ti}") for ti in range(S)]
                for ti in range(S):
                    pv = qps.tile([128, VW], F32, name="pv", tag="qkv")
                    mm = []
                    for dp_i in range(DP):
                        mm.append((hTh[dp_i], wv_h[dp_i]))
                    for dp_i in range(DP):
                        mm.append((hTh[dp_i], wv_l[dp_i]))
                    for dp_i in range(DP):
                        mm.append((hTl[dp_i], wv_h[dp_i]))
                    for mi, (ht_t, wt) in enumerate(mm):
                        nc.tensor.matmul(
                            pv[:], ht_t[:, :, ti * 128:(ti + 1) * 128],
                            wt[:],
                            start=(mi == 0), stop=(mi == len(mm) - 1),
                            perf_mode=DR)
                    nc.vector.tensor_add(vt[ti][:], pv[:], bvb[:])
                # --- attention per head pair ---
                for i, fc in enumerate(fcs):
                    for qb in range(2):
                        if qb == 0:
                            schunks = list(range(S))
                        else:
                            schunks = list(range(QBC, 3 * QBC))
                        ctx_ps = [cps.tile([65, NT], F32, name=f"ctx{hh}",
                                           tag=f"ctx{hh}") for hh in range(2)]
                        nsc = len(schunks)
                        for idx, sc in enumerate(schunks):
                            # mask: (kind, index); 0=none,1=tri,2=scalar
                            if qb == 0:
                                if sc < QBC:
                                    mk = (1, sc)
                                elif sc >= S - QBC:
                                    mk = (2, sc - (S - QBC))
                                else:
                                    mk = (0, 0)
                            else:
                                if sc < 2 * QBC:
                                    mk = (1, sc - QBC)
                                else:
                                    mk = (2, QBC + (sc - 2 * QBC))
                            coff = mk[1] * 128 if mk[0] == 1 else 0
                            ncols = NT - coff
                            sps_t = sps.tile([128, 2, NT], F32,
                                             name="sc", tag="sc")
                            e2 = ep.tile([128, 2, NT], BF16, name="e", tag="e")
                            for hh in range(2):
                                rows = slice(hh * HDf, (hh + 1) * HDf)
                                ssl = _ab('scmm', slice(coff, NT))
                                nc.tensor.matmul(
                                    sps_t[:, hh, ssl],
                                    kT[i][rows, :, sc * 128:(sc + 1) * 128],
                                    qT[i][rows, :,
                                          slice(qb * NT + ssl.start,
                                                qb * NT + ssl.stop)],
                                    start=True, stop=True,
                                    perf_mode=DR,
                                    tile_position=(hh * HDf, 0))
                            ebias = cms[:, mk[1]:mk[1] + 1] \
                                if mk[0] == 2 else zbias[:]
                            esl = _ab('exp', slice(coff, NT))
                            nc.scalar.activation(
                                e2[:, :, esl], sps_t[:, :, esl],
                                AF.Exp, bias=ebias)
                            if mk[0] == 1 and 'tri' not in ABLATE:
                                nc.vector.tensor_mul(
                                    e2[:, :, coff:], e2[:, :, coff:],
                                    tri[mk[1]][:, coff:].unsqueeze(1)
                                    .to_broadcast([128, 2, ncols]))
                            csl = _ab('ctxmm', slice(coff, NT))
                            for hh in range(2):
                                nc.tensor.matmul(
                                    ctx_ps[hh][:, csl],
                                    vt[sc][:, (i * 2 + hh) * 65:
                                           (i * 2 + hh) * 65 + 65],
                                    e2[:, hh, csl],
                                    start=(idx == 0), stop=(idx == nsc - 1),
                                    skip_group_check=True)
                        for hh in range(2):
                            zrow = zp.tile([1, NT], F32, name="zrow",
                                           tag="zrow")
                            nc.vector.tensor_scalar_mul(
                                zrow[:], ctx_ps[hh][64:65, :], 1.0 / CXS)
                            rz = zp.tile([1, NT], F32, name="rz", tag="rz")
                            nc.vector.reciprocal(rz[:], zrow[:])
                            zb = zbp.tile([128, NT], F32, name="zb", tag="zb")
                            nc.gpsimd.partition_broadcast(zb[:], rz[:])
                            rows = slice(hh * HDf, (hh + 1) * HDf)
                            cf = ep.tile([128, NT], BF16, name="cf", tag="cf")
                            nc.vector.tensor_mul(cf[rows], ctx_ps[hh][0:64, :],
                                                 zb[rows])
                            dst_h = cth[fc // 2][rows, fc % 2,
                                                 qb * NT:(qb + 1) * NT]
                            dst_l = ctl[fc // 2][rows, fc % 2,
                                                 qb * NT:(qb + 1) * NT]
                            nc.gpsimd.tensor_copy(dst_h, cf[rows])
                            if 'losub' not in ABLATE:
                                nc.vector.tensor_tensor(
                                    dst_l, cf[rows], dst_h,
                                    op=mybir.AluOpType.subtract)

        hT_stack.close()

        # ---------------- Phase 3: projection + fused LN2 ---------------
        with tc.tile_pool(name="proj_sb", bufs=1) as prp, \
             tc.tile_pool(name="proj_x", bufs=3) as pxp, \
             tc.tile_pool(name="proj_o", bufs=4) as pop, \
             tc.tile_pool(name="ln2s", bufs=8) as lsp2, \
             tc.tile_pool(name="ln2h", bufs=4) as lph2, \
             tc.tile_pool(name="projps", bufs=4, space=bass.MemorySpace.PSUM) as pps, \
             tc.tile_pool(name="ln2p", bufs=4, space=bass.MemorySpace.PSUM) as lpp2:
            bo_row = prp.tile([1, Dm], F32, name="bo_row", tag="bo_row")
            nc.sync.dma_start(bo_row[:], bo_d.ap())
            bob = prp.tile([128, Dm], F32, name="bob", tag="bob")
            nc.gpsimd.partition_broadcast(bob[:], bo_row[:])

            wo_h, wo_l = [], []
            for j in range(FC // 2):
                for lst, dram in ((wo_h, woh_d), (wo_l, wol_d)):
                    t = prp.tile([128, 2, Dm], F8, name=f"wo{j}",
                                 tag=f"wo{dram.name}{j}")
                    nc.gpsimd.dma_start(t[:], dram.ap()[j])
                    lst.append(t)
            for ti in range(TOC):
                xo = pxp.tile([128, Dm], F32, name="xo", tag="xo")
                nc.sync.dma_start(xo[:], xr[ti])
                x1t = pop.tile([128, Dm], BF16, name="x1t", tag="x1t")
                for oc in range(OC):
                    ppt = pps.tile([128, NO], F32, name="ppt", tag="ppt")
                    mm = []
                    for j in range(FC // 2):
                        mm.append((cth[j], wo_h[j]))
                    for j in range(FC // 2):
                        mm.append((ctl[j], wo_h[j]))
                    for j in range(FC // 2):
                        mm.append((cth[j], wo_l[j]))
                    psl = _ab('projmm', slice(oc * NO, (oc + 1) * NO))
                    for mi, (ct, wt) in enumerate(mm):
                        nc.tensor.matmul(
                            ppt[:, 0:psl.stop - psl.start],
                            ct[:, :, ti * 128:(ti + 1) * 128],
                            wt[:, :, psl],
                            start=(mi == 0), stop=(mi == len(mm) - 1),
                            perf_mode=DR)
                    cols = slice(oc * NO, (oc + 1) * NO)
                    pt1 = pop.tile([128, NO], BF16, name="pt1", tag="pt1")
                    nc.scalar.activation(pt1[:], ppt[:], AF.Identity,
                                         scale=1.0 / (WS * CXS))
                    nc.vector.tensor_add(x1t[:, cols], pt1[:], xo[:, cols])
                    nc.gpsimd.tensor_add(x1t[:, cols], x1t[:, cols],
                                         bob[:, cols])
                    # x1b = x1 + b2 (residual for the FFN tail)
                    nc.gpsimd.tensor_add(x1b[ti][:, cols], x1t[:, cols],
                                         b2t[:, cols])
                # fused LN2 on the freshly built x1 tile
                nsub = max(1, Dm // 512)
                st6 = lsp2.tile([128, nsub, 6], F32, name="st6b", tag="st6b")
                for sb_i in range(nsub):
                    nc.vector.bn_stats(
                        st6[:, sb_i, :],
                        x1t[:, sb_i * (Dm // nsub):(sb_i + 1) * (Dm // nsub)])
                agg = lsp2.tile([128, 2], F32, name="aggb", tag="aggb")
                nc.vector.bn_aggr(agg[:], st6[:])
                veps = lsp2.tile([128, 1], F32, name="vepsb", tag="vepsb")
                nc.vector.tensor_scalar_add(veps[:], agg[:, 1:2], EPS)
                std = lsp2.tile([128, 1], F32, name="stdb", tag="stdb")
                nc.scalar.sqrt(std[:], veps[:])
                rstd = lsp2.tile([128, 1], F32, name="rstdb", tag="rstdb")
                nc.vector.reciprocal(rstd[:], std[:])
                nmr = lsp2.tile([128, 1], F32, name="nmrb", tag="nmrb")
                nc.vector.tensor_scalar(nmr[:], agg[:, 0:1], rstd[:], -1.0,
                                        op0=mybir.AluOpType.mult,
                                        op1=mybir.AluOpType.mult)
                hb = lph2.tile([128, Dm], BF16, name="hb", tag="hb")
                nc.scalar.activation(hb[:], x1t[:], AF.Identity,
                                     bias=nmr[:], scale=rstd[:])
                ps2 = lpp2.tile([128, DP, 2, 128], BF16, name="tps2",
                                tag="tps2")
                for dc in range(DC):
                    nc.tensor.transpose(ps2[:, dc // 2, dc % 2, :],
                                        hb[:, dc * 128:(dc + 1) * 128],
                                        ident[:])
                tc_sl = slice(ti * 128, (ti + 1) * 128)
                dst_h = h2ha[:, :, :, tc_sl]
                dst_l = h2la[:, :, :, tc_sl]
                nc.scalar.copy(dst_h, ps2[:])
                nc.vector.tensor_tensor(dst_l, ps2[:], dst_h,
                                        op=mybir.AluOpType.subtract)
        ctx_stack.close()

        # ---------------- Phase 4: FFN ----------------------------------
        with tc.tile_pool(name="ffn_sb", bufs=1) as fp, \
             tc.tile_pool(name="ffn_w1", bufs=2) as w1p, \
             tc.tile_pool(name="ffn_w2", bufs=1) as w2p, \
             tc.tile_pool(name="ffn_b1", bufs=4) as b1p, \
             tc.tile_pool(name="ffn_ff", bufs=2) as ffp, \
             tc.tile_pool(name="ffn_out", bufs=2) as fop, \
             tc.tile_pool(name="ffps", bufs=3, space=bass.MemorySpace.PSUM) as fps, \
             tc.tile_pool(name="outps", bufs=3, space=bass.MemorySpace.PSUM) as ops:
            NJP = NG * GFP
            ffh = [fp.tile([128, 2, TOWN], F8, name=f"ffh{j}", tag=f"ffh{j}")
                   for j in range(NJP)]
            ffl = [fp.tile([128, 2, TOWN], F8, name=f"ffl{j}", tag=f"ffl{j}")
                   for j in range(NJP)]
            w2_h = []
            w2_l = []
            for g in range(NG):
                for jp in range(GFP):
                    for lst, dram in ((w2_h, w2h_d), (w2_l, w2l_d)):
                        t = w2p.tile([128, 2, Dm], F8,
                                     name=f"w2t{g}_{jp}",
                                     tag=f"w2{dram.name}{g}_{jp}")
                        nc.gpsimd.dma_start(t[:], dram.ap()[g, jp])
                        lst.append(t)
            b1all = fp.tile([128, DFFm // 128], F32, name="b1all",
                            tag="b1all")
            nc.gpsimd.dma_start(b1all[:], b1_d.ap())
            for g in range(NG):
                w1_h, w1_l = [], []
                for dp_i in range(DP):
                    for lst, dram in ((w1_h, w1h_d), (w1_l, w1l_d)):
                        t = w1p.tile([128, 2, DFFm // NG], F8,
                                     name=f"w1g{dp_i}",
                                     tag=f"w1{dram.name}{dp_i}")
                        nc.gpsimd.dma_start(t[:], dram.ap()[g, dp_i])
                        lst.append(t)
                for j in range(GFC):
                    gf = g * GFC + j
                    jp_g = g * GFP + j // 2
                    b1t = b1all[:, gf:gf + 1]
                    ws = slice(j * 128, (j + 1) * 128)
                    for tb in range(TOWN // NT):
                        fpt = fps.tile([128, NT], F32, name="fpt", tag="fpt")
                        mm = []
                        for dp_i in range(DP):
                            mm.append((w1_h[dp_i], h2h[dp_i]))
                        for dp_i in range(DP):
                            mm.append((w1_l[dp_i], h2h[dp_i]))
                        for dp_i in range(DP):
                            mm.append((w1_h[dp_i], h2l[dp_i]))
                        fsl = _ab('ffnmm', slice(tb * NT, (tb + 1) * NT))
                        for mi, (wt, ht_t) in enumerate(mm):
                            nc.tensor.matmul(
                                fpt[:, 0:fsl.stop - fsl.start], wt[:, :, ws],
                                ht_t[:, :, fsl],
                                start=(mi == 0), stop=(mi == len(mm) - 1),
                                perf_mode=DR)
                        dst_h = ffh[jp_g][:, j % 2, tb * NT:(tb + 1) * NT]
                        dst_l = ffl[jp_g][:, j % 2, tb * NT:(tb + 1) * NT]
                        nc.scalar.activation(dst_h, fpt[:], AF.Relu,
                                             bias=b1t, scale=FFS / WS)
                        if 'ffl' not in ABLATE:
                            ft2 = ffp.tile([128, NT], BF16, name="ft2",
                                           tag="ft2")
                            nc.scalar.activation(ft2[:], fpt[:], AF.Relu,
                                                 bias=b1t, scale=FFS / WS)
                            nc.vector.tensor_tensor(
                                dst_l, ft2[:], dst_h,
                                op=mybir.AluOpType.subtract)
            for ti in range(TOC):
                for oc in range(OC):
                    opt = ops.tile([128, NO], F32, name="opt", tag="opt")
                    mm = []
                    for jp in range(NJP):
                        mm.append((ffh[jp], w2_h[jp]))
                    for jp in range(NJP):
                        mm.append((ffl[jp], w2_h[jp]))
                    for jp in range(NJP):
                        mm.append((ffh[jp], w2_l[jp]))
                    osl = _ab('ffnmm', slice(oc * NO, (oc + 1) * NO))
                    for mi, (ft, wt) in enumerate(mm):
                        nc.tensor.matmul(
                            opt[:, 0:osl.stop - osl.start],
                            ft[:, :, ti * 128:(ti + 1) * 128],
                            wt[:, :, osl],
                            start=(mi == 0), stop=(mi == len(mm) - 1),
                            perf_mode=DR)
                    cols = slice(oc * NO, (oc + 1) * NO)
                    tmp2 = fop.tile([128, NO], BF16, name="tmp2", tag="tmp2")
                    nc.scalar.activation(tmp2[:], opt[:], AF.Identity,
                                         scale=1.0 / (WS * FFS))
                    ot = fop.tile([128, NO], F32, name="ot", tag="ot")
                    nc.vector.tensor_add(ot[:], tmp2[:], x1b[ti][:, cols])
                    nc.sync.dma_start(outr[ti][:, cols], ot[:])
        h2_stack.close()
    nc.compile()
    return nc


# ---------------------------------------------------------------------------
# host-side input preparation
# ---------------------------------------------------------------------------

def _hilo(w16):
    hi = np.asarray(w16, np.float32).astype(E4)
    lo = (np.asarray(w16, np.float32) - hi.astype(np.float32)).astype(E4)
    return hi, lo


def prepare_shared(cfg, Wq, Wk, Wv, Wo, bo, W1, b1, W2, b2, g1, be1, g2, be2):
    c = derive(cfg)
    Dm, Hn, DFFm, FC = cfg["D"], cfg["H"], cfg["DFF"], c["FC"]
    DP, NPASS, FCP, NG, GFP = c["DP"], c["NPASS"], c["FCP"], cfg["NG"], c["GFP"]
    scale = 1.0 / np.sqrt(HD)
    wq_f = np.ascontiguousarray(Wq.transpose(1, 0, 2).reshape(Dm, Hn * HD))
    wk_f = np.ascontiguousarray(Wk.transpose(1, 0, 2).reshape(Dm, Hn * HD))
    wv_f = np.ascontiguousarray(Wv.transpose(1, 0, 2).reshape(Dm, Hn * HD))
    wq_e = (g1[:, None] * wq_f) * scale
    wk_e = g1[:, None] * wk_f
    wv_e = g1[:, None] * wv_f
    bq = np.ascontiguousarray(((be1 @ wq_f) * scale).reshape(FC, 128).T)
    bk = np.ascontiguousarray((be1 @ wk_f).reshape(FC, 128).T)
    bv = (be1 @ wv_f).reshape(1, Hn * HD)
    w1_e = g2[:, None] * W1
    b1_e = np.ascontiguousarray((b1 + be2 @ W1).reshape(DFFm // 128, 128).T)

    def qkv_tile(w):
        # [D, F] -> [NPASS, DP, 128, 2, FCP*128]; row d = dp*256 + s*128 + r
        return np.ascontiguousarray(
            w.reshape(DP, 2, 128, NPASS, FCP * 128).transpose(3, 0, 2, 1, 4))

    # v weights get a zero column appended per head; its bias is WS, so the
    # v tiles come out of the matmul+bias with a built-in WS column that
    # accumulates the softmax normalizer (scaled) during the ctx matmul.
    nheads = FCP * 2
    wv_r = wv_e.reshape(DP, 2, 128, NPASS, nheads, HD)
    wv_a = np.concatenate(
        [wv_r, np.zeros((DP, 2, 128, NPASS, nheads, 1), wv_r.dtype)], axis=-1)
    wv_t = wv_a.transpose(3, 0, 2, 1, 4, 5).reshape(
        NPASS, DP, 128, 2, nheads * 65)
    bv_r = bv.reshape(NPASS, nheads, HD)
    bv_a = np.concatenate(
        [bv_r, np.ones((NPASS, nheads, 1), bv_r.dtype)], axis=-1)
    bv_t = bv_a.reshape(NPASS, 1, nheads * 65) * WS

    # wo rows = fc-pair layout [FC//2, 128, 2, Dm]
    wo_t = Wo.reshape(FC // 2, 2, 128, Dm).transpose(0, 2, 1, 3)
    # w1 rows like qkv, cols grouped [NG, DP, 128, 2, DFF/NG]
    w1_t = w1_e.reshape(DP, 2, 128, NG, DFFm // NG).transpose(3, 0, 2, 1, 4)
    # w2 rows f = g*1024 + jp*256 + s*128 + r -> [NG, GFP, 128, 2, Dm]
    w2_t = W2.reshape(NG, GFP, 2, 128, Dm).transpose(0, 1, 3, 2, 4)

    f32c = lambda a: np.ascontiguousarray(a, dtype=np.float32)
    out = dict(
        bq=f32c(bq), bk=f32c(bk), bv=f32c(bv_t),
        bo=f32c(bo.reshape(1, Dm)), b1=f32c(4.0 * b1_e),
        b2=f32c(np.broadcast_to(b2.reshape(1, Dm), (128, Dm))),
        ident=np.eye(128, dtype=np.float32).astype(BF),
        zeros=np.zeros((128, 1), np.float32),
    )
    for name, w in (("wq", qkv_tile(wq_e)), ("wk", qkv_tile(wk_e)),
                    ("wv", wv_t), ("wo", wo_t), ("w1", w1_t), ("w2", w2_t)):
        hi, lo = _hilo(np.asarray(w, np.float32) * WS)
        out[name + "h"] = np.ascontiguousarray(hi)
        out[name + "l"] = np.ascontiguousarray(lo)
    return out


def core_plan(cfg, half):
    """Return (perm, qposA, qposB) token index arrays for one core."""
    QB = cfg["QB"]
    Tf = cfg["T"]
    nb = Tf // QB  # 4 blocks
    if half == 0:
        bA, bB = nb - 1, 0
    else:
        bA, bB = nb - 2, 1
    own = {bA, bB}
    restA = [b for b in range(nb) if b not in own and b < bA]
    restB = [b for b in range(nb) if b not in own and b >= bA]
    blocks = [bA, bB] + restA + restB
    perm = np.concatenate([np.arange(b * QB, (b + 1) * QB) for b in blocks])
    qposA = np.arange(bA * QB, (bA + 1) * QB)
    qposB = np.arange(bB * QB, (bB + 1) * QB)
    return perm, qposA, qposB


def make_masks(cfg, perm, qposA, qposB):
    """tri tiles [QBC,128,NT] bf16; whole-chunk exp-bias scalars (0/-80)."""
    c = derive(cfg)
    QBC, NT, S = c["QBC"], c["NT"], c["S"]
    key = perm
    tri = np.zeros((QBC, 128, NT), np.float32)
    for j in range(QBC):
        ks = key[j * 128:(j + 1) * 128]
        tri[j] = (ks[:, None] <= qposA[None, :]).astype(np.float32)
    cm = np.zeros((2 * QBC, 128, 1), np.float32)
    for j in range(QBC):
        sc = S - QBC + j
        ks = key[sc * 128:(sc + 1) * 128]
        m = ks[:, None] <= qposA[None, :]
        assert m.all() or not m.any(), "chunk not homogeneous"
        cm[j] = 0.0 if m.all() else -80.0
    for j in range(QBC):
        sc = 2 * QBC + j
        ks = key[sc * 128:(sc + 1) * 128]
        m = ks[:, None] <= qposB[None, :]
        assert m.all() or not m.any(), "chunk not homogeneous"
        cm[QBC + j] = 0.0 if m.all() else -80.0
    return tri.astype(BF), cm


_NC_CACHE = {}

# test-harness knobs (ignored in normal grading use)
TRACE = False
TRACE_KWARGS = {}
LAST_RESULT = None


def _get_nc(key, cfg):
    if key not in _NC_CACHE:
        _NC_CACHE[key] = build(cfg)
    return _NC_CACHE[key]


def kernel(x, Wq, Wk, Wv, Wo, bo, W1, b1, W2, b2, g1, be1, g2, be2):
    cfg = FULL_CFG
    c = derive(cfg)
    x = np.asarray(x, np.float32)
    shared = prepare_shared(cfg, np.asarray(Wq), np.asarray(Wk), np.asarray(Wv),
                            np.asarray(Wo), np.asarray(bo), np.asarray(W1),
                            np.asarray(b1), np.asarray(W2), np.asarray(b2),
                            np.asarray(g1), np.asarray(be1), np.asarray(g2),
                            np.asarray(be2))
    nc = _get_nc("full", cfg)
    in_maps = []
    plans = []
    for core in range(N_CORES):
        b, half = core // 2, core % 2
        perm, qposA, qposB = core_plan(cfg, half)
        tri, cm = make_masks(cfg, perm, qposA, qposB)
        m = dict(shared)
        m["x"] = np.ascontiguousarray(x[b][perm], np.float32)
        m["tri"] = tri
        m["cm"] = cm
        in_maps.append(m)
        plans.append((b, perm))
    res = run_bass_kernel_spmd(nc, in_maps, list(range(N_CORES)),
                               trace=TRACE, **TRACE_KWARGS)
    global LAST_RESULT
    LAST_RESULT = res
    out = np.zeros((B, T, D), np.float32)
    TOWN = c["TOWN"]
    for core in range(N_CORES):
        b, perm = plans[core]
        o = res.results[core]["out"]
        out[b][perm[:TOWN]] = o
    return out
